# revision 18
# baseline (speedup 1.0000x reference)
"""Multi-head attention (B=8, S=2048, D=512, H=8, DH=64) on 8 TRN2 NeuronCores.

Strategy: data-parallel over the batch dim — core b computes batch element b
end-to-end (no collectives). Per core, everything is kept transposed
("feature on partitions") so softmax reductions land on the TensorE
contraction axis:

  1. QKV projection with head-interleaved, pre-transposed weights gives
     Q^T, K^T laid out (64h+c, s) and V laid out (s, 64h+c). K^T is
     pre-scaled by SCALE*log2(e) on the host so the score matmuls produce
     log2-domain logits directly.
  2. Scores are computed transposed, S^T[j, i], as K=64 matmuls row-packed
     two-at-a-time into disjoint PE row groups (lo/hi replicas of Q^T/K^T).
  3. 2^y runs out of PSUM into bf16 SBUF, split across TWO engines: 3/4 of
     the [128,1024] units go to ScalarE (func=Exp, scale=ln2); 1/4 go to
     the DVE as a pure-Schraudolph int32 build (standard tensor_scalar,
     (y+127)*2^23 with int32 write-conversion) followed by the
     EXP2_FIX3_ANT custom op (mantissa extract via mask tensor + full
     quadratic correction; 0.6% max pointwise error which washes out in
     the softmax ratio). The tensor_scalar releases the PSUM tile as fast
     as ScalarE's exp does, so the psA rotation never stalls on the DVE.
  4. O^T[c, i] = sum_j Vaug[j, c] E^T[j, i] with Vaug = [V | ones]: M=65
     matmuls whose 65th row accumulates the softmax denominator for free.
  5. Normalization: O_un is copied out of PSUM, the denominator row is
     reshaped to (128,16) for a cheap reciprocal, round-trips through
     DRAM for a partition-broadcast, and a GpSimd multiply (VectorE for
     the last head, which is tail-critical) writes the normalized O^T.
  6. Only QK chunk 0 + V precede head 0; QK chunks 1-3 are interleaved
     INSIDE the chunk loops of heads 0-2 so ScalarE/DVE start exp ~40us
     earlier and the PE never idles at head boundaries.
  7. The output projection (+bias) produces out^T which the host
     transposes back.
"""

import numpy as np
import ml_dtypes

B, S, D = 8, 2048, 512
H, DH = 8, 64
INNER = H * DH
SCALE = DH ** -0.5
LOG2E = 1.4426950408889634
LN2 = 0.6931471805599453

N_CORES = 8
NDT = D // 128   # 4 contraction tiles
NSC = S // 128   # 16 s-chunks (j-chunks)
NST = S // 512   # 4 s-tiles

# ---- custom DVE exp2 constants (fit of c0+c1*m+c2*m^2 ~ 2^(m-1)/m) ----
C0_FIT = 1.483488677201297
C1_FIT = -0.729767134486583
C2_FIT = 0.24456257085671512
MASK_I32 = 0x007FFFFF

# Every [128,1024] exp unit is split across both engines: ScalarE exps
# cols [0:S_COLS], the DVE handles cols [S_COLS:1024] via the Schraudolph
# int32 build + EXP2_FIX3_ANT. Both engines pace every PSUM tile, so there
# are no cross-engine substitution bubbles in the psA rotation.
S_COLS = 768
D_COLS = 1024 - S_COLS


def register_exp2_fix3():
    from concourse import dve_ops
    from concourse.dve_spec import (Spec, Src0, Src1, C0, C1, C2, lower,
                                    _has_src1, Bin, AluOp, One)
    from concourse.dve_uop import DveOpSpec

    if "EXP2_FIX3_ANT" in dve_ops.CUSTOM_DVE_SPECS:
        return next(op for op in dve_ops.OPS if op.name == "EXP2_FIX3_ANT")

    f32 = np.float32

    def ref_fix3(in0, in1, s0, s1, imm2):
        u0i = in0.view(np.int32)
        mask = in1.view(np.int32)
        mf = ((u0i & mask) | np.int32(0x3F800000)).view(f32)
        w = (mf * f32(imm2)).astype(f32)
        w2 = (w + f32(s1)).astype(f32)
        w3 = (w2 * mf).astype(f32)
        w4 = (w3 + f32(s0)).astype(f32)
        return (w4 * in0.astype(f32)).astype(f32)

    mb = Bin(AluOp.BITWISE_AND, Src0, Src1)
    mf = Bin(AluOp.BITWISE_OR, mb, One)
    body = ((mf * C2 + C1) * mf + C0) * Src0
    spec = Spec(body=body, reference=ref_fix3)

    row = dve_ops._CUSTOM_DVE_ROW_BASE + len(dve_ops.OPS)
    assert row < 0x20
    dve_ops._SUB_OPCODE_FOR_NAME["EXP2_FIX3_ANT"] = row
    tmp = DveOpSpec(name="EXP2_FIX3_ANT", opcode=row,
                    uops=lower(spec, ver="v3"), rd1_en=_has_src1(spec))
    op = dve_ops.DveOp("EXP2_FIX3_ANT", spec, subdim=False,
                       uops_sha={"v3": tmp.sha("v3")})
    dve_ops.OPS.append(op)
    dve_ops.CUSTOM_DVE_SPECS["EXP2_FIX3_ANT"] = spec
    return op


def _build_kernel():
    import concourse.bass as bass
    import concourse.mybir as mybir
    import concourse.tile as tile
    from concourse import bacc

    OP_FIX3 = register_exp2_fix3()

    bf16 = mybir.dt.bfloat16
    f32 = mybir.dt.float32
    i32 = mybir.dt.int32
    Exp = mybir.ActivationFunctionType.Exp
    ADD = mybir.AluOpType.add
    MULT = mybir.AluOpType.mult

    nc = bacc.Bacc()

    xT = nc.declare_dram_parameter("xT", [D, S], bf16, isOutput=False)
    wq = nc.declare_dram_parameter("wq", [D, INNER], bf16, isOutput=False)
    wk = nc.declare_dram_parameter("wk", [D, INNER], bf16, isOutput=False)
    wv = nc.declare_dram_parameter("wv", [D, INNER], bf16, isOutput=False)
    wo = nc.declare_dram_parameter("wo", [INNER, D], bf16, isOutput=False)
    bo = nc.declare_dram_parameter("bo", [NDT, 128, 1], f32, isOutput=False)
    out = nc.declare_dram_parameter("out", [D, S], f32, isOutput=True)
    den_dram = nc.dram_tensor("den_scratch", [H, S], f32)

    with tile.TileContext(nc) as tc:
        with (
            tc.tile_pool(name="weights", bufs=1) as wpool,
            tc.tile_pool(name="acts", bufs=1) as apool,
            tc.tile_pool(name="et", bufs=8) as epool,
            tc.tile_pool(name="u0", bufs=2) as upool,
            tc.tile_pool(name="small", bufs=2) as spool,
            tc.tile_pool(name="ostage", bufs=2) as opool,
            tc.tile_pool(name="psA", bufs=2, space="PSUM") as psA,
            tc.tile_pool(name="psV", bufs=1, space="PSUM") as psV,
        ):
            # ---- load inputs (x and q/k weights first: they gate head 0) ----
            xT_s = [[wpool.tile([128, S // 2], bf16, name=f"xT{d}_{hf}",
                              tag=f"xT{d}_{hf}") for hf in range(2)]
                    for d in range(NDT)]
            wq_s = [wpool.tile([128, INNER], bf16, name=f"wq{d}", tag=f"wq{d}")
                    for d in range(NDT)]
            wk_s = [wpool.tile([128, INNER], bf16, name=f"wk{d}", tag=f"wk{d}")
                    for d in range(NDT)]
            wv_s = [wpool.tile([128, INNER], bf16, name=f"wv{d}", tag=f"wv{d}")
                    for d in range(NDT)]
            wo_s = [wpool.tile([128, D], bf16, name=f"wo{d}", tag=f"wo{d}")
                    for d in range(NDT)]
            bo_s = [wpool.tile([128, 1], f32, name=f"bo{d}", tag=f"bo{d}")
                    for d in range(NDT)]
            for d in range(NDT):
                sl = slice(d * 128, (d + 1) * 128)
                nc.sync.dma_start(out=xT_s[d][0][:], in_=xT[sl, 0:S // 2])
                nc.sync.dma_start(out=wq_s[d][:], in_=wq[sl, :])
                nc.sync.dma_start(out=wk_s[d][:], in_=wk[sl, :])
            for d in range(NDT):
                sl = slice(d * 128, (d + 1) * 128)
                nc.scalar.dma_start(out=xT_s[d][1][:], in_=xT[sl, S // 2:])
            for d in range(NDT):
                sl = slice(d * 128, (d + 1) * 128)
                nc.scalar.dma_start(out=wv_s[d][:], in_=wv[sl, :])
                nc.scalar.dma_start(out=wo_s[d][:], in_=wo[sl, :])
                nc.scalar.dma_start(out=bo_s[d][:], in_=bo[d, :, :])

            # mantissa mask for the DVE exp path
            mask_sb = wpool.tile([128, 1024], i32, name="mask", tag="mask")
            nc.vector.memset(mask_sb[:, :], MASK_I32)

            # ---- QKV projection ----
            qt_lo = [apool.tile([128, S], bf16, name=f"qlo{t}", tag=f"qlo{t}")
                     for t in range(NDT)]
            kt_lo = [apool.tile([128, S], bf16, name=f"klo{t}", tag=f"klo{t}")
                     for t in range(NDT)]
            qt_hi = [apool.tile([128, S], bf16, name=f"qhi{t}", tag=f"qhi{t}")
                     for t in range(NDT)]
            kt_hi = [apool.tile([128, S], bf16, name=f"khi{t}", tag=f"khi{t}")
                     for t in range(NDT)]

            # PE warm-up: junk matmuls during the input-DMA window keep the
            # HAM activity monitor busy so real matmuls start at 2.4 GHz.
            junk_sb = wpool.tile([128, 512], bf16, name="junk", tag="junk")
            nc.vector.memset(junk_sb[:, :], 0.0)
            junk_ps = psV.tile([128, 4 * 512], f32, name="junkps", tag="pv")
            for k in range(16):
                nc.tensor.matmul(
                    junk_ps[:, (k % 4) * 512:(k % 4 + 1) * 512],
                    lhsT=junk_sb[:, 0:128],
                    rhs=junk_sb[:, :],
                )

            def qk_half(w_s, dst, ch, half):
                pa = psA.tile([128, 1024], f32, name="pa", tag="pa")
                for d in range(NDT):
                    for nn in range(2):
                        s0 = nn * 512
                        nc.tensor.matmul(
                            pa[:, nn * 512:(nn + 1) * 512],
                            lhsT=w_s[d][:, ch * 128:(ch + 1) * 128],
                            rhs=xT_s[d][half][:, s0:s0 + 512],
                            start=(d == 0),
                            stop=(d == NDT - 1),
                        )
                nc.vector.tensor_copy(
                    dst[ch][:, half * 1024:(half + 1) * 1024], pa[:, :])

            def swap_halves(t):
                for (lo, hi) in ((qt_lo, qt_hi), (kt_lo, kt_hi)):
                    nc.sync.dma_start(out=hi[t][64:128, :], in_=lo[t][0:64, :])
                    nc.sync.dma_start(out=hi[t][0:64, :], in_=lo[t][64:128, :])

            for half in range(2):
                qk_half(wq_s, qt_lo, 0, half)
            for half in range(2):
                qk_half(wk_s, kt_lo, 0, half)
            swap_halves(0)

            v_aug = [apool.tile([128, H * (DH + 1)], bf16, name=f"va{m}",
                                tag=f"va{m}") for m in range(NSC)]

            def v_round(r):
                pvt = psV.tile([128, 4 * 512], f32, name="pvt", tag="pv")
                for k in range(4):
                    m = 4 * r + k
                    for d in range(NDT):
                        mh, mo = divmod(m, 8)
                        nc.tensor.matmul(
                            pvt[:, k * 512:(k + 1) * 512],
                            lhsT=xT_s[d][mh][:, mo * 128:(mo + 1) * 128],
                            rhs=wv_s[d][:, :],
                            start=(d == 0),
                            stop=(d == NDT - 1),
                        )
                for k in range(4):
                    m = 4 * r + k
                    va = v_aug[m].rearrange("p (h t) -> p h t", t=DH + 1)
                    nc.vector.tensor_copy(
                        va[:, :, 0:DH],
                        pvt[:, k * 512:(k + 1) * 512].rearrange(
                            "p (h t) -> p h t", t=DH),
                    )
                    nc.vector.memset(va[:, :, DH:DH + 1], 1.0)

            for r in range(NSC // 4):
                v_round(r)

            # ---- attention, head by head; QK chunks 1-3 interleave ----
            ot = [apool.tile([128, S], bf16, name=f"ot{t}", tag=f"ot{t}")
                  for t in range(NDT)]
            f32d = f32

            def head(h, fillers=(), finish_prev=None):
                """fillers: list of (jc, closure) to emit after chunk jc's
                exp units (PE filler work for later heads' Q/K).
                finish_prev: closure finishing the previous head's
                normalize, emitted after chunk 2 (so the broadcast DMA it
                waits on has landed and never blocks this head's DVE queue).
                Returns a closure that applies THIS head's normalize."""
                t, p = h // 2, h % 2
                lo_sl = slice(64 * p, 64 * p + 64)
                hi_sl = slice(64 * (1 - p), 64 * (1 - p) + 64)
                pv = psV.tile([128, 4 * 512], f32, name="pvh", tag="pv")
                ets = {}
                fill = dict(fillers)

                def pv_mms(jc):
                    for it in range(NST):
                        et_h = ets[jc][it // 2]
                        nc.tensor.matmul(
                            pv[0:DH + 1, it * 512:(it + 1) * 512],
                            lhsT=v_aug[jc][:, h * (DH + 1):(h + 1) * (DH + 1)],
                            rhs=et_h[:, (it % 2) * 512:(it % 2 + 1) * 512],
                            start=(jc == 0),
                            stop=(jc == NSC - 1),
                        )

                trail = 1
                for jc in range(NSC):
                    ets[jc] = []
                    for half in range(2):
                        pa = psA.tile([128, 1024], f32, name="pa", tag="pa")
                        et = epool.tile([128, 1024], bf16, name="et",
                                        tag="et")
                        ets[jc].append(et)
                        i0, i1 = 2 * half, 2 * half + 1
                        nc.tensor.matmul(
                            pa[:, 0:512],
                            lhsT=kt_lo[t][lo_sl, jc * 128:(jc + 1) * 128],
                            rhs=qt_lo[t][lo_sl, i0 * 512:(i0 + 1) * 512],
                        )
                        nc.tensor.matmul(
                            pa[:, 512:1024],
                            lhsT=kt_hi[t][hi_sl, jc * 128:(jc + 1) * 128],
                            rhs=qt_hi[t][hi_sl, i1 * 512:(i1 + 1) * 512],
                        )
                        nc.scalar.activation(
                            out=et[:, 0:S_COLS], in_=pa[:, 0:S_COLS],
                            func=Exp, scale=LN2)
                        u0 = upool.tile([128, D_COLS], i32, name="u0",
                                        tag="u0")
                        nc.vector.tensor_scalar(
                            out=u0[:], in0=pa[:, S_COLS:1024],
                            scalar1=127.0, scalar2=8388608.0,
                            op0=ADD, op1=MULT)
                        nc.vector._custom_dve(
                            OP_FIX3, out=et[:, S_COLS:1024],
                            in0=u0[:].bitcast(f32d),
                            in1=mask_sb[:, 0:D_COLS].bitcast(f32d),
                            s0=C0_FIT, s1=C1_FIT, imm2=C2_FIT)
                    if jc in fill:
                        fill[jc]()
                    if finish_prev is not None and 3 <= jc <= 6:
                        finish_prev(jc - 3)
                    if jc >= trail:
                        pv_mms(jc - trail)
                for jc in range(NSC - trail, NSC):
                    pv_mms(jc)

                # Decouple normalization from the exp engines: O_un and the
                # denominator row leave PSUM via DMA (no DVE/ScalarE time),
                # the reciprocal runs on a cheap (128,16) reshape, and the
                # DRAM partition-broadcast lands in 4 pieces so each deferred
                # normalize piece waits on at most one piece's transfer.
                oun = spool.tile([DH + 1, S], f32, name="oun", tag="oun")
                nc.scalar.copy(out=oun[:, 0:1024], in_=pv[0:DH + 1, 0:1024])
                nc.vector.tensor_copy(oun[:, 1024:2048],
                                      pv[0:DH + 1, 1024:2048])
                den128 = spool.tile([128, 16], f32, name="den128", tag="d128")
                nc.sync.dma_start(out=den128[:, :], in_=oun[DH:DH + 1, :])
                nc.vector.reciprocal(out=den128[:, :], in_=den128[:, :])
                nc.sync.dma_start(out=den_dram[h, :], in_=den128[:, :])
                bc = spool.tile([64, S], f32, name="bc", tag="bc")
                dd = den_dram[h:h + 1, :]
                for q4 in range(4):
                    cs = slice(q4 * 512, (q4 + 1) * 512)
                    bcast_src = bass.AP(
                        tensor=dd.tensor,
                        offset=dd.offset + q4 * 512,
                        ap=[[0, 64], [1, 512]],
                    )
                    nc.sync.dma_start(out=bc[:, cs], in_=bcast_src)

                def norm_piece(q4):
                    rows = slice(64 * p, 64 * p + 64)
                    cs = slice(q4 * 512, (q4 + 1) * 512)
                    nc.vector.tensor_mul(
                        ot[t][rows, cs], oun[0:DH, cs], bc[:, cs])
                return norm_piece

            # QK chunks 1-2 ride as PE fillers in head 0's chunk loop and
            # chunk 3 in head 1's (heads 2-3 need chunk 1, 4-5 chunk 2,
            # 6-7 chunk 3 — all ready long before use). The exp engines
            # pace those heads, so the filler matmuls are nearly free.
            fills = {0: [], 1: []}
            for c in range(1, NDT):
                host = 0 if c < NDT - 1 else 1
                jcf = 1 + (c - 1) * 4 if host == 0 else 1
                for w_s, dst in ((wq_s, qt_lo), (wk_s, kt_lo)):
                    for hf in range(2):
                        if w_s is wk_s and hf == 1:
                            fills[host].append(
                                (jcf, lambda c=c, hf=hf: (
                                    qk_half(wk_s, kt_lo, c, hf),
                                    swap_halves(c))))
                        else:
                            fills[host].append(
                                (jcf, lambda c=c, w_s=w_s, dst=dst, hf=hf:
                                 qk_half(w_s, dst, c, hf)))
                        jcf += 1
            fin = None
            for h in range(H):
                fin = head(h, fills.get(h, ()), finish_prev=fin)
            for q4 in range(4):
                fin(q4)  # last head's normalize, gates the projection

            # keep the PE warm through the den-path latency window so the
            # output projection runs at 2.4 GHz, not the cold 1.2 GHz
            junk_ps2 = psA.tile([128, 1024], f32, name="junkps2", tag="pa")
            for k in range(12):
                nc.tensor.matmul(
                    junk_ps2[:, (k % 2) * 512:(k % 2 + 1) * 512],
                    lhsT=junk_sb[:, 0:128],
                    rhs=junk_sb[:, :],
                )

            # ---- output projection (psA ping-pong so matmul groups and the
            # bias-add/copy of the previous group overlap) ----
            for ch in range(NDT):
                for half in range(2):
                    stage = opool.tile([128, 1024], f32, name="stage",
                                       tag="stage")
                    # alternate PSUM pools: psV's banks are free once the
                    # last head's O_un is copied out, doubling the number of
                    # projection accumulation groups in flight
                    po_pool = psA if (ch * 2 + half) % 2 == 0 else psV
                    po = po_pool.tile([128, 1024], f32, name="po",
                                      tag="pa" if po_pool is psA else "pv")
                    for st2 in range(2):
                        st = half * 2 + st2
                        for kt in range(NDT):
                            nc.tensor.matmul(
                                po[:, st2 * 512:(st2 + 1) * 512],
                                lhsT=wo_s[kt][:, ch * 128:(ch + 1) * 128],
                                rhs=ot[kt][:, st * 512:(st + 1) * 512],
                                start=(kt == 0),
                                stop=(kt == NDT - 1),
                            )
                    nc.vector.tensor_scalar_add(
                        out=stage[:, :],
                        in0=po[:, :],
                        scalar1=bo_s[ch][:, :],
                    )
                    nc.sync.dma_start(
                        out=out[ch * 128:(ch + 1) * 128,
                                half * 1024:(half + 1) * 1024],
                        in_=stage[:, :],
                    )

    nc.finalize()
    return nc


_NC_CACHE = None


def _get_nc():
    global _NC_CACHE
    if _NC_CACHE is None:
        _NC_CACHE = _build_kernel()
    return _NC_CACHE


def _prep_inputs(x, W_qkv, W_out, b_out):
    bf16 = ml_dtypes.bfloat16
    # head-interleave and transpose the qkv weight: row 192h+{0,64,128}+c of
    # W_qkv is q/k/v row (h, c); regroup to e' = 64h+c and transpose to [d, e']
    w3 = W_qkv.reshape(H, 3, DH, D)
    wq_h = np.ascontiguousarray(w3[:, 0].reshape(INNER, D).T).astype(bf16)
    # fold softmax scale and log2(e) into K so scores are log2-domain logits
    wk_h = np.ascontiguousarray(
        w3[:, 1].reshape(INNER, D).T * np.float32(SCALE * LOG2E)).astype(bf16)
    wv_h = np.ascontiguousarray(w3[:, 2].reshape(INNER, D).T).astype(bf16)
    wo_h = np.ascontiguousarray(W_out.T).astype(bf16)  # [hc, d]
    bo_h = np.ascontiguousarray(b_out.reshape(NDT, 128, 1)).astype(np.float32)
    in_maps = []
    for b in range(N_CORES):
        xT_b = np.ascontiguousarray(x[b].T).astype(bf16)  # [d, s]
        in_maps.append({
            "xT": xT_b, "wq": wq_h, "wk": wk_h, "wv": wv_h,
            "wo": wo_h, "bo": bo_h,
        })
    return in_maps


def kernel(x, W_qkv, W_out, b_out):
    from concourse.bass_utils import run_bass_kernel_spmd

    in_maps = _prep_inputs(x, W_qkv, W_out, b_out)
    nc = _get_nc()
    res = run_bass_kernel_spmd(nc, in_maps, list(range(N_CORES)))
    outs = [res.results[b]["out"].T for b in range(N_CORES)]  # [s, d] each
    return np.ascontiguousarray(np.stack(outs, axis=0)).astype(np.float32)


# revision 20
# speedup vs baseline: 1.2362x; 1.2362x over previous
"""Multi-head attention (B=8, S=2048, D=512, H=8, DH=64) on 8 TRN2 NeuronCores.

Strategy: data-parallel over the batch dim — core b computes batch element b
end-to-end (no collectives). Per core, everything is kept transposed
("feature on partitions") so softmax reductions land on the TensorE
contraction axis:

  1. QKV projection with head-interleaved, pre-transposed weights gives
     Q^T, K^T laid out (64h+c, s) and V laid out (s, 64h+c). K^T is
     pre-scaled by SCALE*log2(e) on the host so the score matmuls produce
     log2-domain logits directly.
  2. Scores are computed transposed, S^T[j, i], as K=64 matmuls row-packed
     two-at-a-time into disjoint PE row groups (lo/hi replicas of Q^T/K^T).
  3. 2^y runs on ScalarE (func=Exp, scale=ln2) straight out of PSUM into
     bf16 SBUF. ScalarE is the kernel's bottleneck engine (~99% busy in
     the attention phase); everything else is kept off its queue.
  4. O^T[c, i] = sum_j Vaug[j, c] E^T[j, i] with Vaug = [V | ones]: M=65
     matmuls whose 65th row accumulates the softmax denominator for free.
  5. Normalization: O_un is copied out of PSUM by the (slack) VectorE, the
     denominator row is reshaped to (128,16) for a cheap reciprocal and
     round-trips DRAM in 4 pieces for a partition-broadcast; the multiply
     that applies it is DEFERRED into the next head's chunk loop so its
     DMA latency never blocks an engine queue. The last head's pieces
     gate the output projection incrementally.
  6. Only QK chunk 0 + V precede head 0: QK chunks 1-2 ride as PE fillers
     inside head 0's chunk loop and chunk 3 inside head 1's, so ScalarE
     starts exp'ing ~30us earlier than a serial projection phase would.
  7. A handful of junk matmuls before the output projection keep the PE's
     HAM activity monitor warm through the last den-path latency window;
     the projection accumulates in PSUM groups alternating between two
     pools (+bias via VectorE) and produces out^T, which the host
     transposes back.
"""

import numpy as np
import ml_dtypes

B, S, D = 8, 2048, 512
H, DH = 8, 64
INNER = H * DH
SCALE = DH ** -0.5
LOG2E = 1.4426950408889634
LN2 = 0.6931471805599453

N_CORES = 8
NDT = D // 128   # 4 contraction tiles
NSC = S // 128   # 16 s-chunks (j-chunks)
NST = S // 512   # 4 s-tiles

# ---- custom DVE exp2 constants (fit of c0+c1*m+c2*m^2 ~ 2^(m-1)/m) ----
C0_FIT = 1.483488677201297
C1_FIT = -0.729767134486583
C2_FIT = 0.24456257085671512
MASK_I32 = 0x007FFFFF

# Every [128,1024] exp unit is split across both engines: ScalarE exps
# cols [0:S_COLS], the DVE handles cols [S_COLS:1024] via the Schraudolph
# int32 build + EXP2_FIX3_ANT. Both engines pace every PSUM tile, so there
# are no cross-engine substitution bubbles in the psA rotation.
S_COLS = 768
D_COLS = 1024 - S_COLS


def register_exp2_fix3():
    from concourse import dve_ops
    from concourse.dve_spec import (Spec, Src0, Src1, C0, C1, C2, lower,
                                    _has_src1, Bin, AluOp, One)
    from concourse.dve_uop import DveOpSpec

    if "EXP2_FIX3_ANT" in dve_ops.CUSTOM_DVE_SPECS:
        return next(op for op in dve_ops.OPS if op.name == "EXP2_FIX3_ANT")

    f32 = np.float32

    def ref_fix3(in0, in1, s0, s1, imm2):
        u0i = in0.view(np.int32)
        mask = in1.view(np.int32)
        mf = ((u0i & mask) | np.int32(0x3F800000)).view(f32)
        w = (mf * f32(imm2)).astype(f32)
        w2 = (w + f32(s1)).astype(f32)
        w3 = (w2 * mf).astype(f32)
        w4 = (w3 + f32(s0)).astype(f32)
        return (w4 * in0.astype(f32)).astype(f32)

    mb = Bin(AluOp.BITWISE_AND, Src0, Src1)
    mf = Bin(AluOp.BITWISE_OR, mb, One)
    body = ((mf * C2 + C1) * mf + C0) * Src0
    spec = Spec(body=body, reference=ref_fix3)

    row = dve_ops._CUSTOM_DVE_ROW_BASE + len(dve_ops.OPS)
    assert row < 0x20
    dve_ops._SUB_OPCODE_FOR_NAME["EXP2_FIX3_ANT"] = row
    tmp = DveOpSpec(name="EXP2_FIX3_ANT", opcode=row,
                    uops=lower(spec, ver="v3"), rd1_en=_has_src1(spec))
    op = dve_ops.DveOp("EXP2_FIX3_ANT", spec, subdim=False,
                       uops_sha={"v3": tmp.sha("v3")})
    dve_ops.OPS.append(op)
    dve_ops.CUSTOM_DVE_SPECS["EXP2_FIX3_ANT"] = spec
    return op


def _build_kernel():
    import concourse.bass as bass
    import concourse.mybir as mybir
    import concourse.tile as tile
    from concourse import bacc

    OP_FIX3 = register_exp2_fix3()

    bf16 = mybir.dt.bfloat16
    f32 = mybir.dt.float32
    i32 = mybir.dt.int32
    Exp = mybir.ActivationFunctionType.Exp
    ADD = mybir.AluOpType.add
    MULT = mybir.AluOpType.mult

    nc = bacc.Bacc()

    xT = nc.declare_dram_parameter("xT", [D, S], bf16, isOutput=False)
    wq = nc.declare_dram_parameter("wq", [D, INNER], bf16, isOutput=False)
    wk = nc.declare_dram_parameter("wk", [D, INNER], bf16, isOutput=False)
    wv = nc.declare_dram_parameter("wv", [D, INNER], bf16, isOutput=False)
    wo = nc.declare_dram_parameter("wo", [INNER, D], bf16, isOutput=False)
    bo = nc.declare_dram_parameter("bo", [NDT, 128, 1], f32, isOutput=False)
    out = nc.declare_dram_parameter("out", [D, S], f32, isOutput=True)
    den_dram = nc.dram_tensor("den_scratch", [H, S], f32)

    with tile.TileContext(nc) as tc:
        with (
            tc.tile_pool(name="weights", bufs=1) as wpool,
            tc.tile_pool(name="acts", bufs=1) as apool,
            tc.tile_pool(name="et", bufs=8) as epool,
            tc.tile_pool(name="small", bufs=2) as spool,
            tc.tile_pool(name="ostage", bufs=2) as opool,
            tc.tile_pool(name="psA", bufs=2, space="PSUM") as psA,
            tc.tile_pool(name="psV", bufs=1, space="PSUM") as psV,
        ):
            # ---- load inputs (x and q/k weights first: they gate head 0) ----
            xT_s = [[wpool.tile([128, S // 2], bf16, name=f"xT{d}_{hf}",
                              tag=f"xT{d}_{hf}") for hf in range(2)]
                    for d in range(NDT)]
            wq_s = [wpool.tile([128, INNER], bf16, name=f"wq{d}", tag=f"wq{d}")
                    for d in range(NDT)]
            wk_s = [wpool.tile([128, INNER], bf16, name=f"wk{d}", tag=f"wk{d}")
                    for d in range(NDT)]
            wv_s = [wpool.tile([128, INNER], bf16, name=f"wv{d}", tag=f"wv{d}")
                    for d in range(NDT)]
            wo_s = [wpool.tile([128, D], bf16, name=f"wo{d}", tag=f"wo{d}")
                    for d in range(NDT)]
            bo_s = [wpool.tile([128, 1], f32, name=f"bo{d}", tag=f"bo{d}")
                    for d in range(NDT)]
            for d in range(NDT):
                sl = slice(d * 128, (d + 1) * 128)
                nc.sync.dma_start(out=xT_s[d][0][:], in_=xT[sl, 0:S // 2])
                nc.sync.dma_start(out=wq_s[d][:], in_=wq[sl, :])
                nc.sync.dma_start(out=wk_s[d][:], in_=wk[sl, :])
            for d in range(NDT):
                sl = slice(d * 128, (d + 1) * 128)
                nc.scalar.dma_start(out=xT_s[d][1][:], in_=xT[sl, S // 2:])
            for d in range(NDT):
                sl = slice(d * 128, (d + 1) * 128)
                nc.scalar.dma_start(out=wv_s[d][:], in_=wv[sl, :])
                nc.scalar.dma_start(out=wo_s[d][:], in_=wo[sl, :])
                nc.scalar.dma_start(out=bo_s[d][:], in_=bo[d, :, :])

            # ---- QKV projection ----
            qt_lo = [apool.tile([128, S], bf16, name=f"qlo{t}", tag=f"qlo{t}")
                     for t in range(NDT)]
            kt_lo = [apool.tile([128, S], bf16, name=f"klo{t}", tag=f"klo{t}")
                     for t in range(NDT)]
            qt_hi = [apool.tile([128, S], bf16, name=f"qhi{t}", tag=f"qhi{t}")
                     for t in range(NDT)]
            kt_hi = [apool.tile([128, S], bf16, name=f"khi{t}", tag=f"khi{t}")
                     for t in range(NDT)]

            # PE warm-up: junk matmuls during the input-DMA window keep the
            # HAM activity monitor busy so real matmuls start at 2.4 GHz.
            junk_sb = wpool.tile([128, 512], bf16, name="junk", tag="junk")
            nc.vector.memset(junk_sb[:, :], 0.0)
            junk_ps = psV.tile([128, 4 * 512], f32, name="junkps", tag="pv")
            for k in range(16):
                nc.tensor.matmul(
                    junk_ps[:, (k % 4) * 512:(k % 4 + 1) * 512],
                    lhsT=junk_sb[:, 0:128],
                    rhs=junk_sb[:, :],
                )

            def qk_half(w_s, dst, ch, half):
                pa = psA.tile([128, 1024], f32, name="pa", tag="pa")
                for d in range(NDT):
                    for nn in range(2):
                        s0 = nn * 512
                        nc.tensor.matmul(
                            pa[:, nn * 512:(nn + 1) * 512],
                            lhsT=w_s[d][:, ch * 128:(ch + 1) * 128],
                            rhs=xT_s[d][half][:, s0:s0 + 512],
                            start=(d == 0),
                            stop=(d == NDT - 1),
                        )
                nc.vector.tensor_copy(
                    dst[ch][:, half * 1024:(half + 1) * 1024], pa[:, :])

            def swap_halves(t):
                for (lo, hi) in ((qt_lo, qt_hi), (kt_lo, kt_hi)):
                    nc.sync.dma_start(out=hi[t][64:128, :], in_=lo[t][0:64, :])
                    nc.sync.dma_start(out=hi[t][0:64, :], in_=lo[t][64:128, :])

            for half in range(2):
                qk_half(wq_s, qt_lo, 0, half)
            for half in range(2):
                qk_half(wk_s, kt_lo, 0, half)
            swap_halves(0)

            v_aug = [apool.tile([128, H * (DH + 1)], bf16, name=f"va{m}",
                                tag=f"va{m}") for m in range(NSC)]

            def v_round(r):
                pvt = psV.tile([128, 4 * 512], f32, name="pvt", tag="pv")
                for k in range(4):
                    m = 4 * r + k
                    for d in range(NDT):
                        mh, mo = divmod(m, 8)
                        nc.tensor.matmul(
                            pvt[:, k * 512:(k + 1) * 512],
                            lhsT=xT_s[d][mh][:, mo * 128:(mo + 1) * 128],
                            rhs=wv_s[d][:, :],
                            start=(d == 0),
                            stop=(d == NDT - 1),
                        )
                for k in range(4):
                    m = 4 * r + k
                    va = v_aug[m].rearrange("p (h t) -> p h t", t=DH + 1)
                    nc.vector.tensor_copy(
                        va[:, :, 0:DH],
                        pvt[:, k * 512:(k + 1) * 512].rearrange(
                            "p (h t) -> p h t", t=DH),
                    )
                    nc.vector.memset(va[:, :, DH:DH + 1], 1.0)

            for r in range(NSC // 4):
                v_round(r)

            # ---- attention, head by head; QK chunks 1-3 interleave ----
            ot = [apool.tile([128, S], bf16, name=f"ot{t}", tag=f"ot{t}")
                  for t in range(NDT)]
            f32d = f32

            def head(h, fillers=(), finish_prev=None):
                """fillers: list of (jc, closure) to emit after chunk jc's
                exp units (PE filler work for later heads' Q/K).
                finish_prev: closure finishing the previous head's
                normalize, emitted after chunk 2 (so the broadcast DMA it
                waits on has landed and never blocks this head's DVE queue).
                Returns a closure that applies THIS head's normalize."""
                t, p = h // 2, h % 2
                lo_sl = slice(64 * p, 64 * p + 64)
                hi_sl = slice(64 * (1 - p), 64 * (1 - p) + 64)
                pv = psV.tile([128, 4 * 512], f32, name="pvh", tag="pv")
                ets = {}
                fill = dict(fillers)

                def pv_mms(jc):
                    for it in range(NST):
                        et_h = ets[jc][it // 2]
                        nc.tensor.matmul(
                            pv[0:DH + 1, it * 512:(it + 1) * 512],
                            lhsT=v_aug[jc][:, h * (DH + 1):(h + 1) * (DH + 1)],
                            rhs=et_h[:, (it % 2) * 512:(it % 2 + 1) * 512],
                            start=(jc == 0),
                            stop=(jc == NSC - 1),
                        )

                trail = 1
                for jc in range(NSC):
                    ets[jc] = []
                    for half in range(2):
                        pa = psA.tile([128, 1024], f32, name="pa", tag="pa")
                        et = epool.tile([128, 1024], bf16, name="et",
                                        tag="et")
                        ets[jc].append(et)
                        i0, i1 = 2 * half, 2 * half + 1
                        nc.tensor.matmul(
                            pa[:, 0:512],
                            lhsT=kt_lo[t][lo_sl, jc * 128:(jc + 1) * 128],
                            rhs=qt_lo[t][lo_sl, i0 * 512:(i0 + 1) * 512],
                        )
                        nc.tensor.matmul(
                            pa[:, 512:1024],
                            lhsT=kt_hi[t][hi_sl, jc * 128:(jc + 1) * 128],
                            rhs=qt_hi[t][hi_sl, i1 * 512:(i1 + 1) * 512],
                        )
                        nc.scalar.activation(
                            out=et[:, :], in_=pa[:, :],
                            func=Exp, scale=LN2)
                    if jc in fill:
                        fill[jc]()
                    if finish_prev is not None and 3 <= jc <= 6:
                        finish_prev(jc - 3)
                    if jc >= trail:
                        pv_mms(jc - trail)
                for jc in range(NSC - trail, NSC):
                    pv_mms(jc)

                # Decouple normalization from the exp engines: O_un and the
                # denominator row leave PSUM via DMA (no DVE/ScalarE time),
                # the reciprocal runs on a cheap (128,16) reshape, and the
                # DRAM partition-broadcast lands in 4 pieces so each deferred
                # normalize piece waits on at most one piece's transfer.
                oun = spool.tile([DH + 1, S], f32, name="oun", tag="oun")
                nc.vector.tensor_copy(oun[:, :], pv[0:DH + 1, :])
                den128 = spool.tile([128, 16], f32, name="den128", tag="d128")
                nc.sync.dma_start(out=den128[:, :], in_=oun[DH:DH + 1, :])
                nc.vector.reciprocal(out=den128[:, :], in_=den128[:, :])
                nc.sync.dma_start(out=den_dram[h, :], in_=den128[:, :])
                bc = spool.tile([64, S], f32, name="bc", tag="bc")
                dd = den_dram[h:h + 1, :]
                for q4 in range(4):
                    cs = slice(q4 * 512, (q4 + 1) * 512)
                    bcast_src = bass.AP(
                        tensor=dd.tensor,
                        offset=dd.offset + q4 * 512,
                        ap=[[0, 64], [1, 512]],
                    )
                    nc.sync.dma_start(out=bc[:, cs], in_=bcast_src)

                def norm_piece(q4):
                    rows = slice(64 * p, 64 * p + 64)
                    cs = slice(q4 * 512, (q4 + 1) * 512)
                    nc.vector.tensor_mul(
                        ot[t][rows, cs], oun[0:DH, cs], bc[:, cs])
                return norm_piece

            # QK chunks 1-2 ride as PE fillers in head 0's chunk loop and
            # chunk 3 in head 1's (heads 2-3 need chunk 1, 4-5 chunk 2,
            # 6-7 chunk 3 — all ready long before use). The exp engines
            # pace those heads, so the filler matmuls are nearly free.
            fills = {0: [], 1: []}
            for c in range(1, NDT):
                host = 0 if c < NDT - 1 else 1
                jcf = 1 + (c - 1) * 4 if host == 0 else 1
                for w_s, dst in ((wq_s, qt_lo), (wk_s, kt_lo)):
                    for hf in range(2):
                        if w_s is wk_s and hf == 1:
                            fills[host].append(
                                (jcf, lambda c=c, hf=hf: (
                                    qk_half(wk_s, kt_lo, c, hf),
                                    swap_halves(c))))
                        else:
                            fills[host].append(
                                (jcf, lambda c=c, w_s=w_s, dst=dst, hf=hf:
                                 qk_half(w_s, dst, c, hf)))
                        jcf += 1
            fin = None
            for h in range(H):
                fin = head(h, fills.get(h, ()), finish_prev=fin)
            for q4 in range(4):
                fin(q4)  # last head's normalize, gates the projection

            # keep the PE warm through the den-path latency window so the
            # output projection runs at 2.4 GHz, not the cold 1.2 GHz
            junk_ps2 = psA.tile([128, 1024], f32, name="junkps2", tag="pa")
            for k in range(12):
                nc.tensor.matmul(
                    junk_ps2[:, (k % 2) * 512:(k % 2 + 1) * 512],
                    lhsT=junk_sb[:, 0:128],
                    rhs=junk_sb[:, :],
                )

            # ---- output projection (psA ping-pong so matmul groups and the
            # bias-add/copy of the previous group overlap) ----
            for ch in range(NDT):
                for half in range(2):
                    stage = opool.tile([128, 1024], f32, name="stage",
                                       tag="stage")
                    # alternate PSUM pools: psV's banks are free once the
                    # last head's O_un is copied out, doubling the number of
                    # projection accumulation groups in flight
                    po_pool = psA if (ch * 2 + half) % 2 == 0 else psV
                    po = po_pool.tile([128, 1024], f32, name="po",
                                      tag="pa" if po_pool is psA else "pv")
                    for st2 in range(2):
                        st = half * 2 + st2
                        for kt in range(NDT):
                            nc.tensor.matmul(
                                po[:, st2 * 512:(st2 + 1) * 512],
                                lhsT=wo_s[kt][:, ch * 128:(ch + 1) * 128],
                                rhs=ot[kt][:, st * 512:(st + 1) * 512],
                                start=(kt == 0),
                                stop=(kt == NDT - 1),
                            )
                    nc.vector.tensor_scalar_add(
                        out=stage[:, :],
                        in0=po[:, :],
                        scalar1=bo_s[ch][:, :],
                    )
                    nc.sync.dma_start(
                        out=out[ch * 128:(ch + 1) * 128,
                                half * 1024:(half + 1) * 1024],
                        in_=stage[:, :],
                    )

    nc.finalize()
    return nc


_NC_CACHE = None


def _get_nc():
    global _NC_CACHE
    if _NC_CACHE is None:
        _NC_CACHE = _build_kernel()
    return _NC_CACHE


def _prep_inputs(x, W_qkv, W_out, b_out):
    bf16 = ml_dtypes.bfloat16
    # head-interleave and transpose the qkv weight: row 192h+{0,64,128}+c of
    # W_qkv is q/k/v row (h, c); regroup to e' = 64h+c and transpose to [d, e']
    w3 = W_qkv.reshape(H, 3, DH, D)
    wq_h = np.ascontiguousarray(w3[:, 0].reshape(INNER, D).T).astype(bf16)
    # fold softmax scale and log2(e) into K so scores are log2-domain logits
    wk_h = np.ascontiguousarray(
        w3[:, 1].reshape(INNER, D).T * np.float32(SCALE * LOG2E)).astype(bf16)
    wv_h = np.ascontiguousarray(w3[:, 2].reshape(INNER, D).T).astype(bf16)
    wo_h = np.ascontiguousarray(W_out.T).astype(bf16)  # [hc, d]
    bo_h = np.ascontiguousarray(b_out.reshape(NDT, 128, 1)).astype(np.float32)
    in_maps = []
    for b in range(N_CORES):
        xT_b = np.ascontiguousarray(x[b].T).astype(bf16)  # [d, s]
        in_maps.append({
            "xT": xT_b, "wq": wq_h, "wk": wk_h, "wv": wv_h,
            "wo": wo_h, "bo": bo_h,
        })
    return in_maps


def kernel(x, W_qkv, W_out, b_out):
    from concourse.bass_utils import run_bass_kernel_spmd

    in_maps = _prep_inputs(x, W_qkv, W_out, b_out)
    nc = _get_nc()
    res = run_bass_kernel_spmd(nc, in_maps, list(range(N_CORES)))
    outs = [res.results[b]["out"].T for b in range(N_CORES)]  # [s, d] each
    return np.ascontiguousarray(np.stack(outs, axis=0)).astype(np.float32)
